# revision 13
# baseline (speedup 1.0000x reference)
"""MoE (top-2 of 8 experts, B=8192, D=2048) on 8 Trainium2 NeuronCores.

Strategy (expert-parallel, per sharding hint): the host computes the gate
softmax + top-2 routing (float64 numpy; rank-2/3 margins are ~3e-5 so the
selection matches any f32 reference platform), dispatches each token's rows
to its experts' cores, and each core computes
    y_e = relu(x_e @ W[e].T + b[e]) * gate_scale
for its gathered tokens as a mixed fp16/fp8 tiled matmul on the PE array.
The host then scatter-adds the (at most 2) expert contributions per token.

Precision split (measured rel-err 1.6e-2 vs the 2e-2 gate): contraction
dims 0:1536 run fp16 (12 K=128 chunks); dims 1536:2048 run e4m3 fp8 as
2 DoubleRow chunks (K=256 each, 2x MACs/cycle) -> 14 streams per
(m,n)-panel instead of 16 (-12.5% PE time).  To dodge e4m3's subnormal
floor (W sigma ~0.022 < 2^-6) both planes are pre-scaled on the host
(x*2^5, W*2^10, products *2^15); the 2^-15 unscale is folded into the
pre-scaled bias (*2^15) and gate scale (*2^-15), so the epilogue is the
same two DVE ops as pure fp16.

Load balance (flex panels): expert loads are 2048 +- ~100 tokens, so
padding every core to the max (17 m-tiles = 68 panels) wastes ~6us.
Instead every core runs exactly 16 own m-tiles (64 panels) plus 2 flex
panels whose weights/tokens/bias/scale are per-core INPUTS: the 12
overflow panels (3 experts x 4 n-panels of their 17th tile) are spread
across the 16 flex slots, unused slots run zeros -> 66 panels/core.

Schedule (inherited from the fp16-only 255us kernel, at the fp16
streaming bound of 216 ns per [K128,N512] stream):
- Start: first-needed ~2.4MB interleaved across the two HWDGE queues in
  strict need-order chunks; phase 1 opens with m0/m1 interleaved at
  kd-pair granularity; ~40 garbage warmup matmuls bridge the ~7us engine
  preamble AND the ~15us wt0 delivery window, holding the HAM clock gate
  (1.2->2.4GHz) busy whatever phase its free-running window starts in
  (a >3.4us-idle re-throttle costs ~3.5us, measured).
- wt[1..3]/w8[1..3]/flex inputs are paced on the gpsimd SWDGE queue,
  each pinned behind a phase-1 epilogue output via a 1-row WAW copy
  (into the region the DMA then overwrites).
- Phase 2 runs m-outer, accumulating the n=1..3 epilogues into one
  [P,1536] tile -> a single 384KB DMA per m with 3KB lines.  The flex
  panels run last, the final one as two sequential 256-col groups so
  the post-matmul tail is one [P,256] epilogue + 128KB DMA + the fixed
  ~3us end barrier.
"""

import math

import numpy as np
import ml_dtypes

B, D, E, TOP_K = 8192, 2048, 8, 2
N_CORES = 8
P = 128
KD16 = 12          # fp16 contraction chunks (dims 0:1536)
NP8 = 4            # fp8 k-planes of 128 (dims 1536:2048) -> 2 DoubleRow chunks
D16 = KD16 * P     # 1536
NT = 4
NSZ = D // NT      # 512 output columns per psum tile
M_TILES = 16       # own tiles per core (= 2048 tokens)
FLEX = 2           # flex panel slots per core
WARMUP_MM = 20

SX = 32.0          # x pre-scale (2^5)
SW = 1024.0        # W pre-scale (2^10)
SPROD = SX * SW    # 2^15

_F16 = np.float16
_F8 = ml_dtypes.float8_e4m3   # TRN FP8_EXP4: max +-240

_nc_cache = {}


def _routing(x, Wg, bg):
    """Gate softmax + top-2 in float64; returns (idx [B,2] int, vals [B,2] f32)."""
    logits = x.astype(np.float64) @ Wg.astype(np.float64).T + bg.astype(np.float64)
    logits -= logits.max(-1, keepdims=True)
    eL = np.exp(logits)
    gate = eL / eL.sum(-1, keepdims=True)
    order = np.argsort(-gate, axis=-1, kind="stable")
    idx = order[:, :TOP_K]
    vals = np.take_along_axis(gate, idx, -1).astype(np.float32)
    return idx, vals


def _build():
    """Build + compile the per-core Bass kernel (16 own tiles + 2 flex)."""
    import concourse.mybir as mybir
    import concourse.tile as tile
    from concourse import bacc

    nc = bacc.Bacc("TRN2", target_bir_lowering=False)
    m_tiles = M_TILES
    C = m_tiles * P
    xt = nc.dram_tensor("xt", [P, m_tiles, KD16, P], mybir.dt.float16, kind="ExternalInput")
    x8 = nc.dram_tensor("x8", [P, m_tiles, NP8, P], mybir.dt.float8e4, kind="ExternalInput")
    wt = nc.dram_tensor("wt", [P, NT, KD16, NSZ], mybir.dt.float16, kind="ExternalInput")
    w8 = nc.dram_tensor("w8", [P, NT, NP8, NSZ], mybir.dt.float8e4, kind="ExternalInput")
    bias = nc.dram_tensor("bias", [P, D], mybir.dt.float16, kind="ExternalInput")
    scale = nc.dram_tensor("scale", [P, m_tiles], mybir.dt.float32, kind="ExternalInput")
    xf = nc.dram_tensor("xf", [P, FLEX, KD16, P], mybir.dt.float16, kind="ExternalInput")
    xf8 = nc.dram_tensor("xf8", [P, FLEX, NP8, P], mybir.dt.float8e4, kind="ExternalInput")
    wf = nc.dram_tensor("wf", [P, FLEX, KD16, NSZ], mybir.dt.float16, kind="ExternalInput")
    wf8 = nc.dram_tensor("wf8", [P, FLEX, NP8, NSZ], mybir.dt.float8e4, kind="ExternalInput")
    biasf = nc.dram_tensor("biasf", [P, FLEX, NSZ], mybir.dt.float16, kind="ExternalInput")
    scalef = nc.dram_tensor("scalef", [P, FLEX], mybir.dt.float32, kind="ExternalInput")
    y = nc.dram_tensor("y", [C, D], mybir.dt.float16, kind="ExternalOutput")
    yf = nc.dram_tensor("yf", [FLEX, P, NSZ], mybir.dt.float16, kind="ExternalOutput")

    DR = mybir.MatmulPerfMode.DoubleRow

    with tile.TileContext(nc) as tc:
        with (
            # Raw (dependency-untracked) SBUF for the warmup operands: the
            # garbage contents are never read back, and having no writer
            # lets the first warmup matmul issue right at PE-preamble end.
            nc.sbuf_tensor([P, 640], mybir.dt.float16) as warm,
            tc.tile_pool(name="wp", bufs=1) as wp,
            tc.tile_pool(name="w8p", bufs=1) as w8p,
            tc.tile_pool(name="xp", bufs=1) as xp,
            tc.tile_pool(name="x8p", bufs=1) as x8p,
            tc.tile_pool(name="fp", bufs=1) as fpp,
            tc.tile_pool(name="cp", bufs=1) as cp,
            tc.tile_pool(name="op", bufs=8) as op_,
            tc.tile_pool(name="oy", bufs=4) as oyp,
            tc.tile_pool(name="pp", bufs=8, space="PSUM") as pp,
        ):
            # Everything latency-critical rides the two HWDGE queues in
            # FIFO order; the gpsimd SWDGE queue stays empty until the y
            # writes (which are gated by epilogue deps) so it can never
            # starve the early loads on the shared DMA engines.
            xts = [None] * m_tiles
            x8s = [None] * m_tiles

            wts = [None] * NT
            w8s = [None] * NT
            wts[0] = wp.tile([P, KD16, NSZ], mybir.dt.float16, tag="wt0", name="wt_sb0")
            w8s[0] = w8p.tile([P, NP8, NSZ], mybir.dt.float8e4, tag="w80", name="w8_sb0")

            def load_xt_on(m, eng):
                t = xp.tile([P, KD16, P], mybir.dt.float16, tag=f"xt{m}", name=f"xt_sb{m}")
                eng.dma_start(t[:], xt[:, m])
                xts[m] = t
                t8 = x8p.tile([P, NP8, P], mybir.dt.float8e4, tag=f"x8{m}", name=f"x8_sb{m}")
                eng.dma_start(t8[:], x8[:, m])
                x8s[m] = t8

            # Both HWDGE queues carry the start-critical pieces in
            # time-of-need order (FIFO per queue); m0/m1 interleave at
            # kd-PAIR granularity and the queues deliver in matching
            # round-robin ~384KB rounds, so no single PE data-stall can
            # exceed the ~3.4us HAM idle window (a longer stall drops the
            # PE clock to 1.2GHz for ~10us -- measured, costs ~5us).
            xts[0] = xp.tile([P, KD16, P], mybir.dt.float16, tag="xt0", name="xt_sb0")
            xts[1] = xp.tile([P, KD16, P], mybir.dt.float16, tag="xt1", name="xt_sb1")
            x8s[0] = x8p.tile([P, NP8, P], mybir.dt.float8e4, tag="x80", name="x8_sb0")
            x8s[1] = x8p.tile([P, NP8, P], mybir.dt.float8e4, tag="x81", name="x8_sb1")
            for r in range(KD16 // 2):
                sl = slice(2 * r, 2 * r + 2)
                qa, qb = (nc.scalar, nc.sync) if r % 2 == 0 else (nc.sync, nc.scalar)
                qa.dma_start(wts[0][:, sl], wt[:, 0, sl])
                qb.dma_start(xts[0][:, sl], xt[:, 0, sl])
                qb.dma_start(xts[1][:, sl], xt[:, 1, sl])
            nc.scalar.dma_start(w8s[0][:], w8[:, 0])
            nc.sync.dma_start(x8s[0][:], x8[:, 0])
            nc.sync.dma_start(x8s[1][:], x8[:, 1])
            load_xt_on(2, nc.scalar)
            bias_sb = cp.tile([P, D], mybir.dt.float16, tag="bias", name="bias_sb")
            scale_sb = cp.tile([P, m_tiles], mybir.dt.float32, tag="scale", name="scale_sb")
            for m in range(3, m_tiles):
                load_xt_on(m, nc.sync if m % 2 == 1 else nc.scalar)
                if m == 3:
                    # bias/scale aren't needed until the first epilogue
                    # (~20us); keep them out of the critical start window.
                    nc.sync.dma_start(bias_sb[:], bias[:])
                    nc.sync.dma_start(scale_sb[:], scale[:])
            for n in range(1, NT):
                wts[n] = wp.tile([P, KD16, NSZ], mybir.dt.float16, tag=f"wt{n}", name=f"wt_sb{n}")
                w8s[n] = w8p.tile([P, NP8, NSZ], mybir.dt.float8e4, tag=f"w8{n}", name=f"w8_sb{n}")

            # flex input tiles
            xf_sb = fpp.tile([P, FLEX, KD16, P], mybir.dt.float16, tag="xf", name="xf_sb")
            xf8_sb = fpp.tile([P, FLEX, NP8, P], mybir.dt.float8e4, tag="xf8", name="xf8_sb")
            wf_sb = fpp.tile([P, FLEX, KD16, NSZ], mybir.dt.float16, tag="wf", name="wf_sb")
            wf8_sb = fpp.tile([P, FLEX, NP8, NSZ], mybir.dt.float8e4, tag="wf8", name="wf8_sb")
            biasf_sb = fpp.tile([P, FLEX, NSZ], mybir.dt.float16, tag="biasf", name="biasf_sb")
            scalef_sb = fpp.tile([P, FLEX], mybir.dt.float32, tag="scalef", name="scalef_sb")

            # Paced gpsimd loads: wt/w8 chunks for n=1..3 (4 each), then
            # flex inputs in 3 bundles.  Each is pinned behind a phase-1
            # epilogue output via a 1-row WAW copy INTO the region the DMA
            # then overwrites.
            def load_chunk(k, gate=None):
                n, c = k // 4 + 1, k % 4
                if k < 12:
                    if gate is not None:
                        if c == 3:
                            nc.vector.tensor_copy(w8s[n][0:1, 0, 0:NSZ], gate[0:1, 0:NSZ])
                        else:
                            nc.vector.tensor_copy(wts[n][0:1, 4 * c, 0:NSZ], gate[0:1, 0:NSZ])
                    if c == 3:
                        nc.gpsimd.dma_start(w8s[n][:], w8[:, n])
                    else:
                        sl = slice(c * 4, (c + 1) * 4)
                        nc.gpsimd.dma_start(wts[n][:, sl], wt[:, n, sl])
                elif k == 12:
                    if gate is not None:
                        nc.vector.tensor_copy(wf_sb[0:1, 0, 0, 0:NSZ], gate[0:1, 0:NSZ])
                    nc.gpsimd.dma_start(wf_sb[:, 0], wf[:, 0])
                    nc.gpsimd.dma_start(wf8_sb[:, 0], wf8[:, 0])
                elif k == 13:
                    if gate is not None:
                        nc.vector.tensor_copy(wf_sb[0:1, 1, 0, 0:NSZ], gate[0:1, 0:NSZ])
                    nc.gpsimd.dma_start(wf_sb[:, 1], wf[:, 1])
                    nc.gpsimd.dma_start(wf8_sb[:, 1], wf8[:, 1])
                elif k == 14:
                    if gate is not None:
                        nc.vector.tensor_copy(xf_sb[0:1, 0, 0, 0:P], gate[0:1, 0:P])
                    nc.gpsimd.dma_start(xf_sb[:], xf[:])
                    nc.gpsimd.dma_start(xf8_sb[:], xf8[:])
                    nc.gpsimd.dma_start(biasf_sb[:], biasf[:])
                    nc.gpsimd.dma_start(scalef_sb[:], scalef[:])

            N_CHUNKS = 15

            # PE warmup: bridge the engine preamble until the first wt[0]
            # chunk lands (~11us), keeping the HAM clock busy.
            wps = pp.tile([P, NSZ], mybir.dt.float32, tag="ps", name="warmps")
            for _w in range(2 * WARMUP_MM):
                nc.tensor.matmul(wps[:, 0:NSZ // 2], warm[:, 0:P],
                                 warm[:, P:P + NSZ // 2],
                                 start=True, stop=True)

            def mm_f8(ps, xt8ap, w8ap):
                """The 2 fp8 DoubleRow chunks closing a panel group."""
                for j in range(NP8 // 2):
                    nc.tensor.matmul(
                        ps, xt8ap[:, 2 * j:2 * j + 2, :], w8ap[:, 2 * j:2 * j + 2, :],
                        start=False, stop=(j == NP8 // 2 - 1), perf_mode=DR,
                    )

            def epilogue_n0(ps, m):
                ot = op_.tile([P, NSZ], mybir.dt.float32, tag="ot", name="ot")
                nc.vector.tensor_tensor(
                    ot[:], ps[:], bias_sb[:, 0:NSZ], mybir.AluOpType.add
                )
                ot16 = op_.tile([P, NSZ], mybir.dt.float16, tag="ot16", name="ot16")
                nc.vector.tensor_scalar(
                    ot16[:], ot[:], scale_sb[:, m:m + 1], 0.0,
                    mybir.AluOpType.mult, mybir.AluOpType.max,
                )
                nc.gpsimd.dma_start(y[m * P:(m + 1) * P, 0:NSZ], ot16[:])
                return ot16

            # Phase 1: n=0 sweep.  m0/m1 interleave their fp16 kds at PAIR
            # granularity so the matmuls consume wt0/xt pieces in delivery
            # order during the DMA ramp (each stall stays well under the
            # HAM idle window); m2+ run as plain accumulation groups.
            psA = pp.tile([P, NSZ], mybir.dt.float32, tag="ps", name="ps")
            psB = pp.tile([P, NSZ], mybir.dt.float32, tag="ps", name="ps")
            for r in range(KD16 // 2):
                for kd in (2 * r, 2 * r + 1):
                    nc.tensor.matmul(psA[:], xts[0][:, kd], wts[0][:, kd],
                                     start=(kd == 0), stop=False)
                for kd in (2 * r, 2 * r + 1):
                    nc.tensor.matmul(psB[:], xts[1][:, kd], wts[0][:, kd],
                                     start=(kd == 0), stop=False)
            mm_f8(psA[:], x8s[0], w8s[0])
            ot16 = epilogue_n0(psA, 0)
            load_chunk(0, gate=ot16)
            mm_f8(psB[:], x8s[1], w8s[0])
            ot16 = epilogue_n0(psB, 1)
            load_chunk(1, gate=ot16)
            for m in range(2, m_tiles):
                ps = pp.tile([P, NSZ], mybir.dt.float32, tag="ps", name="ps")
                for kd in range(KD16):
                    nc.tensor.matmul(
                        ps[:], xts[m][:, kd], wts[0][:, kd],
                        start=(kd == 0), stop=False,
                    )
                mm_f8(ps[:], x8s[m], w8s[0])
                ot16 = epilogue_n0(ps, m)
                if m < N_CHUNKS:
                    load_chunk(m, gate=ot16)
            for k in range(m_tiles, N_CHUNKS):
                load_chunk(k)

            # Phase 2: m-outer / n-inner; 3 psum banks per m; epilogues
            # accumulate into one [P, 3*NSZ] fp16 tile -> single 384KB DMA
            # with 3KB lines.
            for m in range(m_tiles):
                pss = [pp.tile([P, NSZ], mybir.dt.float32, tag="ps", name="ps")
                       for _ in range(NT - 1)]
                for kd in range(KD16):
                    for j in range(NT - 1):
                        nc.tensor.matmul(
                            pss[j][:], xts[m][:, kd], wts[j + 1][:, kd],
                            start=(kd == 0), stop=False,
                        )
                for p8 in range(NP8 // 2):
                    for j in range(NT - 1):
                        nc.tensor.matmul(
                            pss[j][:], x8s[m][:, 2 * p8:2 * p8 + 2, :],
                            w8s[j + 1][:, 2 * p8:2 * p8 + 2, :],
                            start=False, stop=(p8 == NP8 // 2 - 1), perf_mode=DR,
                        )
                oty = oyp.tile([P, (NT - 1) * NSZ], mybir.dt.float16, tag="oty", name="oty")
                for j in range(NT - 1):
                    n = j + 1
                    ot = op_.tile([P, NSZ], mybir.dt.float32, tag="ot", name="ot")
                    nc.vector.tensor_tensor(
                        ot[:], pss[j][:], bias_sb[:, n * NSZ:(n + 1) * NSZ],
                        mybir.AluOpType.add
                    )
                    nc.vector.tensor_scalar(
                        oty[:, j * NSZ:(j + 1) * NSZ], ot[:], scale_sb[:, m:m + 1], 0.0,
                        mybir.AluOpType.mult, mybir.AluOpType.max,
                    )
                eng = nc.sync if m % 2 == 0 else nc.gpsimd
                eng.dma_start(y[m * P:(m + 1) * P, NSZ:D], oty[:])

            # Flex panels close the kernel.  Slot 0 runs as one [P,512]
            # group; slot 1 (the very last work) as two sequential 256-col
            # groups, so the first half's epilogue + DMA overlap the second
            # half's matmuls and the post-matmul tail is minimal.
            s = 0
            ps = pp.tile([P, NSZ], mybir.dt.float32, tag="ps", name="ps")
            for kd in range(KD16):
                nc.tensor.matmul(
                    ps[:], xf_sb[:, s, kd], wf_sb[:, s, kd],
                    start=(kd == 0), stop=False,
                )
            mm_f8(ps[:], xf8_sb[:, s], wf8_sb[:, s])
            ot = op_.tile([P, NSZ], mybir.dt.float32, tag="ot", name="ot")
            nc.vector.tensor_tensor(
                ot[:], ps[:], biasf_sb[:, s], mybir.AluOpType.add
            )
            ot16 = op_.tile([P, NSZ], mybir.dt.float16, tag="ot16", name="ot16")
            nc.vector.tensor_scalar(
                ot16[:], ot[:], scalef_sb[:, s:s + 1], 0.0,
                mybir.AluOpType.mult, mybir.AluOpType.max,
            )
            nc.sync.dma_start(yf[s], ot16[:])

            s = 1
            for c in range(2):
                cs = slice(c * (NSZ // 2), (c + 1) * (NSZ // 2))
                ps = pp.tile([P, NSZ], mybir.dt.float32, tag="ps", name="ps")
                for kd in range(KD16):
                    nc.tensor.matmul(
                        ps[:, 0:NSZ // 2], xf_sb[:, s, kd],
                        wf_sb[:, s, kd, cs],
                        start=(kd == 0), stop=False,
                    )
                for j in range(NP8 // 2):
                    nc.tensor.matmul(
                        ps[:, 0:NSZ // 2], xf8_sb[:, s, 2 * j:2 * j + 2, :],
                        wf8_sb[:, s, 2 * j:2 * j + 2, cs],
                        start=False, stop=(j == NP8 // 2 - 1), perf_mode=DR,
                    )
                otc = op_.tile([P, NSZ // 2], mybir.dt.float32, tag="otc", name="otc")
                nc.vector.tensor_tensor(
                    otc[:], ps[:, 0:NSZ // 2], biasf_sb[:, s, cs],
                    mybir.AluOpType.add
                )
                otc16 = op_.tile([P, NSZ // 2], mybir.dt.float16, tag="otc16", name="otc16")
                nc.vector.tensor_scalar(
                    otc16[:], otc[:], scalef_sb[:, s:s + 1], 0.0,
                    mybir.AluOpType.mult, mybir.AluOpType.max,
                )
                nc.sync.dma_start(yf[s, :, cs], otc16[:])

    nc.compile()
    return nc


def _get_nc():
    if 0 not in _nc_cache:
        _nc_cache[0] = _build()
    return _nc_cache[0]


def _q8(a):
    return np.clip(np.asarray(a, np.float32), -240.0, 240.0).astype(_F8)


def _xt_blocks(Xp, m_tiles):
    """[C, D] float32 (pre-scaled) -> fp16 [P,m,12,P] and fp8 [P,m,4,P]."""
    xt_np = np.ascontiguousarray(
        Xp[:, :D16].astype(_F16).reshape(m_tiles, P, KD16, P).transpose(3, 0, 2, 1)
    )
    x8_np = np.ascontiguousarray(
        _q8(Xp[:, D16:]).reshape(m_tiles, P, NP8, P).transpose(3, 0, 2, 1)
    )
    return xt_np, x8_np


def _prep_inputs(x, W, b, idx, vals):
    """Per-core input maps: blocked fp16/fp8 xT/wT layouts + flex panels."""
    token_lists = []
    counts = []
    for e in range(E):
        tok = np.where((idx == e).any(axis=1))[0]
        token_lists.append(tok)
        counts.append(len(tok))
    C = M_TILES * P

    def s_of(e, tok):
        s = np.zeros(len(tok), dtype=np.float32)
        for k in range(TOP_K):
            sel = idx[tok, k] == e
            s[sel] = vals[tok[sel], k]
        return s

    # flex jobs: overflow tiles (tokens beyond C), one job per n-panel
    flex_jobs = []  # (expert, tok_overflow, n)
    for e in range(E):
        ov = token_lists[e][C:]
        assert len(ov) <= P, "overflow beyond one tile not supported"
        if len(ov):
            for n in range(NT):
                flex_jobs.append((e, ov, n))
    assert len(flex_jobs) <= N_CORES * FLEX, "too many flex jobs"
    per_core_jobs = [[] for _ in range(N_CORES)]
    for i, job in enumerate(flex_jobs):
        per_core_jobs[i % N_CORES].append(job)

    # precompute per-expert weight blocks
    in_maps = []
    wt_cache = {}
    for e in range(E):
        We = W[e].astype(np.float32) * SW
        wt_np = np.ascontiguousarray(
            We[:, :D16].astype(_F16).reshape(NT, NSZ, KD16, P).transpose(3, 0, 2, 1)
        )
        w8_np = np.ascontiguousarray(
            We[:, D16:].reshape(NT, NSZ, NP8, P).transpose(3, 0, 2, 1)
        )
        wt_cache[e] = (wt_np, _q8(w8_np))

    for e in range(E):
        tok = token_lists[e][:C]
        cnt = len(tok)
        Xp = np.zeros((C, D), dtype=np.float32)
        Xp[:cnt] = x[tok] * SX
        xt_np, x8_np = _xt_blocks(Xp, M_TILES)
        wt_np, w8_np = wt_cache[e]
        bias_np = np.ascontiguousarray(
            np.broadcast_to(b[e] * SPROD, (P, D)).astype(_F16))
        s_tok = np.zeros(C, dtype=np.float32)
        s_tok[:cnt] = s_of(e, tok)
        scale_np = np.ascontiguousarray((s_tok / SPROD).reshape(M_TILES, P).T)

        # flex slots for this core
        xf_np = np.zeros((P, FLEX, KD16, P), dtype=_F16)
        xf8_np = np.zeros((P, FLEX, NP8, P), dtype=_F8)
        wf_np = np.zeros((P, FLEX, KD16, NSZ), dtype=_F16)
        wf8_np = np.zeros((P, FLEX, NP8, NSZ), dtype=_F8)
        biasf_np = np.zeros((P, FLEX, NSZ), dtype=_F16)
        scalef_np = np.zeros((P, FLEX), dtype=np.float32)
        for slot, (fe, ov, n) in enumerate(per_core_jobs[e]):
            Xo = np.zeros((P, D), dtype=np.float32)
            Xo[:len(ov)] = x[ov] * SX
            fxt, fx8 = _xt_blocks(Xo, 1)
            xf_np[:, slot] = fxt[:, 0]
            xf8_np[:, slot] = fx8[:, 0]
            fwt, fw8 = wt_cache[fe]
            wf_np[:, slot] = fwt[:, n]
            wf8_np[:, slot] = fw8[:, n]
            biasf_np[:, slot] = np.broadcast_to(
                (b[fe][n * NSZ:(n + 1) * NSZ] * SPROD).astype(_F16), (P, NSZ))
            so = np.zeros(P, dtype=np.float32)
            so[:len(ov)] = s_of(fe, ov)
            scalef_np[:, slot] = so / SPROD

        in_maps.append({
            "xt": xt_np, "x8": x8_np, "wt": wt_np, "w8": w8_np,
            "bias": bias_np, "scale": scale_np,
            "xf": np.ascontiguousarray(xf_np), "xf8": np.ascontiguousarray(xf8_np),
            "wf": np.ascontiguousarray(wf_np), "wf8": np.ascontiguousarray(wf8_np),
            "biasf": np.ascontiguousarray(biasf_np),
            "scalef": np.ascontiguousarray(scalef_np),
        })
    return in_maps, token_lists, counts, per_core_jobs


def kernel(x, W, b, Wg, bg):
    from concourse.bass_utils import run_bass_kernel_spmd

    x = np.asarray(x, dtype=np.float32)
    W = np.asarray(W, dtype=np.float32)
    b = np.asarray(b, dtype=np.float32)
    Wg = np.asarray(Wg, dtype=np.float32)
    bg = np.asarray(bg, dtype=np.float32)

    idx, vals = _routing(x, Wg, bg)
    in_maps, token_lists, counts, per_core_jobs = _prep_inputs(x, W, b, idx, vals)
    nc = _get_nc()
    res = run_bass_kernel_spmd(nc, in_maps, core_ids=list(range(N_CORES)))

    C = M_TILES * P
    out = np.zeros((B, D), dtype=np.float32)
    for e in range(E):
        ye = res.results[e]["y"]
        tok = token_lists[e][:C]
        out[tok] += ye[:len(tok)].astype(np.float32)
        yfe = res.results[e]["yf"]
        for slot, (fe, ov, n) in enumerate(per_core_jobs[e]):
            out[ov, n * NSZ:(n + 1) * NSZ] += yfe[slot][:len(ov)].astype(np.float32)
    return out


# revision 14
# speedup vs baseline: 1.0036x; 1.0036x over previous
"""MoE (top-2 of 8 experts, B=8192, D=2048) on 8 Trainium2 NeuronCores.

Strategy (expert-parallel, per sharding hint): the host computes the gate
softmax + top-2 routing (float64 numpy; rank-2/3 margins are ~3e-5 so the
selection matches any f32 reference platform), dispatches each token's rows
to its experts' cores, and each core computes
    y_e = relu(x_e @ W[e].T + b[e]) * gate_scale
for its gathered tokens as a mixed fp16/fp8 tiled matmul on the PE array.
The host then scatter-adds the (at most 2) expert contributions per token.

Precision split (measured rel-err 1.5e-2 vs the 2e-2 gate): contraction
dims 0:1536 run fp16 (12 K=128 chunks); dims 1536:2048 run e4m3 fp8 as
2 DoubleRow chunks (K=256 each, 2x MACs/cycle) -> 14 streams per
(m,n)-tile instead of 16 (-12.5% PE time).  To dodge e4m3's subnormal
floor (W sigma ~0.022 < 2^-6) both planes are pre-scaled on the host
(x*2^5, W*2^10, products *2^15); the 2^-15 unscale is folded into the
pre-scaled bias (*2^15) and gate scale (*2^-15), so the epilogue is the
same two DVE ops as pure fp16.

Schedule (inherited from the fp16-only 255us kernel, which was at the
fp16 streaming bound of 216 ns per [K128,N512] stream):
- Steady state: 17m x 4n x (12 fp16 + 2 fp8 DoubleRow) streams.
- Start: first-needed ~2.4MB interleaved across the two HWDGE queues in
  strict need-order 0.2-0.4MB chunks; phase 1 opens with m0/m1
  interleaved at half-K granularity; ~20 garbage warmup matmuls bridge
  the engine preamble and hold the HAM clock gate busy.
- wt[1..3]/w8[1..3] are paced in 12 chunks on the gpsimd SWDGE queue,
  each pinned behind a phase-1 epilogue output via a 1-row WAW copy.
- Phase 2 runs m-outer, accumulating the n=1..3 epilogues into one
  [P,1536] tile -> a single 384KB DMA per m with 3KB lines.  The final
  m-tile runs its three panels sequentially with immediate per-panel
  epilogues, the last one as two sequential 256-col groups.
"""

import math

import numpy as np
import ml_dtypes

B, D, E, TOP_K = 8192, 2048, 8, 2
N_CORES = 8
P = 128
KD16 = 12          # fp16 contraction chunks (dims 0:1536)
NP8 = 4            # fp8 k-planes of 128 (dims 1536:2048) -> 2 DoubleRow chunks
D16 = KD16 * P     # 1536
NT = 4
NSZ = D // NT      # 512 output columns per psum tile
WARMUP_MM = 10

SX = 32.0          # x pre-scale (2^5)
SW = 1024.0        # W pre-scale (2^10)
SPROD = SX * SW    # 2^15

_F16 = np.float16
_F8 = ml_dtypes.float8_e4m3   # TRN FP8_EXP4: max +-240

_nc_cache = {}


def _routing(x, Wg, bg):
    """Gate softmax + top-2 in float64; returns (idx [B,2] int, vals [B,2] f32)."""
    logits = x.astype(np.float64) @ Wg.astype(np.float64).T + bg.astype(np.float64)
    logits -= logits.max(-1, keepdims=True)
    eL = np.exp(logits)
    gate = eL / eL.sum(-1, keepdims=True)
    order = np.argsort(-gate, axis=-1, kind="stable")
    idx = order[:, :TOP_K]
    vals = np.take_along_axis(gate, idx, -1).astype(np.float32)
    return idx, vals


def _build(m_tiles):
    """Build + compile the per-core Bass kernel for C = m_tiles*128 tokens."""
    import concourse.mybir as mybir
    import concourse.tile as tile
    from concourse import bacc

    nc = bacc.Bacc("TRN2", target_bir_lowering=False)
    C = m_tiles * P
    xt = nc.dram_tensor("xt", [P, m_tiles, KD16, P], mybir.dt.float16, kind="ExternalInput")
    x8 = nc.dram_tensor("x8", [P, m_tiles, NP8, P], mybir.dt.float8e4, kind="ExternalInput")
    wt = nc.dram_tensor("wt", [P, NT, KD16, NSZ], mybir.dt.float16, kind="ExternalInput")
    w8 = nc.dram_tensor("w8", [P, NT, NP8, NSZ], mybir.dt.float8e4, kind="ExternalInput")
    bias = nc.dram_tensor("bias", [P, D], mybir.dt.float16, kind="ExternalInput")
    scale = nc.dram_tensor("scale", [P, m_tiles], mybir.dt.float32, kind="ExternalInput")
    y = nc.dram_tensor("y", [C, D], mybir.dt.float16, kind="ExternalOutput")

    DR = mybir.MatmulPerfMode.DoubleRow

    with tile.TileContext(nc) as tc:
        with (
            # Raw (dependency-untracked) SBUF for the warmup operands: the
            # garbage contents are never read back, and having no writer
            # lets the first warmup matmul issue right at PE-preamble end.
            nc.sbuf_tensor([P, 640], mybir.dt.float16) as warm,
            tc.tile_pool(name="wp", bufs=1) as wp,
            tc.tile_pool(name="w8p", bufs=1) as w8p,
            tc.tile_pool(name="xp", bufs=1) as xp,
            tc.tile_pool(name="x8p", bufs=1) as x8p,
            tc.tile_pool(name="cp", bufs=1) as cp,
            tc.tile_pool(name="op", bufs=10) as op_,
            tc.tile_pool(name="oy", bufs=4) as oyp,
            tc.tile_pool(name="pp", bufs=8, space="PSUM") as pp,
        ):
            # Everything latency-critical rides the two HWDGE queues in
            # FIFO order; the gpsimd SWDGE queue stays empty until the y
            # writes (which are gated by epilogue deps) so it can never
            # starve the early loads on the shared DMA engines.
            xts = [None] * m_tiles
            x8s = [None] * m_tiles

            wts = [None] * NT
            w8s = [None] * NT
            wts[0] = wp.tile([P, KD16, NSZ], mybir.dt.float16, tag="wt0", name="wt_sb0")
            w8s[0] = w8p.tile([P, NP8, NSZ], mybir.dt.float8e4, tag="w80", name="w8_sb0")

            def load_xt_on(m, eng):
                t = xp.tile([P, KD16, P], mybir.dt.float16, tag=f"xt{m}", name=f"xt_sb{m}")
                eng.dma_start(t[:], xt[:, m])
                xts[m] = t
                t8 = x8p.tile([P, NP8, P], mybir.dt.float8e4, tag=f"x8{m}", name=f"x8_sb{m}")
                eng.dma_start(t8[:], x8[:, m])
                x8s[m] = t8

            # Both HWDGE queues carry the start-critical pieces in
            # time-of-need order (FIFO per queue); m0/m1 interleave their
            # fp16 kd halves, with the fp8 operands arriving after the
            # second halves.
            xts[0] = xp.tile([P, KD16, P], mybir.dt.float16, tag="xt0", name="xt_sb0")
            xts[1] = xp.tile([P, KD16, P], mybir.dt.float16, tag="xt1", name="xt_sb1")
            x8s[0] = x8p.tile([P, NP8, P], mybir.dt.float8e4, tag="x80", name="x8_sb0")
            x8s[1] = x8p.tile([P, NP8, P], mybir.dt.float8e4, tag="x81", name="x8_sb1")
            # SP's first packet lands ~1us before ACT's, so the two
            # first-needed pieces go one per queue, then strict
            # need-order interleave.
            nc.sync.dma_start(xts[0][:, 0:6], xt[:, 0, 0:6])
            nc.sync.dma_start(wts[0][:, 2:4], wt[:, 0, 2:4])
            nc.sync.dma_start(wts[0][:, 4:6], wt[:, 0, 4:6])
            nc.sync.dma_start(xts[0][:, 6:12], xt[:, 0, 6:12])
            nc.sync.dma_start(x8s[0][:], x8[:, 0])
            nc.sync.dma_start(w8s[0][:], w8[:, 0])

            nc.scalar.dma_start(wts[0][:, 0:2], wt[:, 0, 0:2])
            nc.scalar.dma_start(xts[1][:, 0:6], xt[:, 1, 0:6])
            nc.scalar.dma_start(wts[0][:, 6:9], wt[:, 0, 6:9])
            nc.scalar.dma_start(wts[0][:, 9:12], wt[:, 0, 9:12])
            nc.scalar.dma_start(xts[1][:, 6:12], xt[:, 1, 6:12])
            nc.scalar.dma_start(x8s[1][:], x8[:, 1])
            load_xt_on(2, nc.scalar)
            bias_sb = cp.tile([P, D], mybir.dt.float16, tag="bias", name="bias_sb")
            scale_sb = cp.tile([P, m_tiles], mybir.dt.float32, tag="scale", name="scale_sb")
            for m in range(3, m_tiles):
                load_xt_on(m, nc.sync if m % 2 == 1 else nc.scalar)
                if m == 3:
                    # bias/scale aren't needed until the first epilogue
                    # (~20us); keep them out of the critical start window.
                    nc.sync.dma_start(bias_sb[:], bias[:])
                    nc.sync.dma_start(scale_sb[:], scale[:])
            for n in range(1, NT):
                wts[n] = wp.tile([P, KD16, NSZ], mybir.dt.float16, tag=f"wt{n}", name=f"wt_sb{n}")
                w8s[n] = w8p.tile([P, NP8, NSZ], mybir.dt.float8e4, tag=f"w8{n}", name=f"w8_sb{n}")

            # wt1..3 + w8_1..3 chunk list: 12 pieces, emitted on the gpsimd
            # engine interleaved with the dep-gated phase-1 y triggers,
            # which paces the SWDGE queue so it never starves the xt stream.
            _wt_chunks = [(n, c) for n in range(1, NT) for c in range(4)]

            def load_wt_chunk(k, gate=None):
                n, c = _wt_chunks[k]
                if gate is not None:
                    # Pin: a 1-row copy from the (dep-gated) epilogue output
                    # into the chunk creates a WAW dependency the scheduler
                    # cannot hoist the DMA past -- this paces the wt[1..3]
                    # stream behind phase-1 progress so it never floods the
                    # DMA engines during the latency-critical start window.
                    # The copy MUST land inside the region the DMA then
                    # overwrites (w8s full tile / wts chunk c's kd range).
                    if c == 3:
                        nc.vector.tensor_copy(w8s[n][0:1, 0, 0:NSZ], gate[0:1, 0:NSZ])
                    else:
                        nc.vector.tensor_copy(wts[n][0:1, 4 * c, 0:NSZ], gate[0:1, 0:NSZ])
                if c == 3:
                    nc.gpsimd.dma_start(w8s[n][:], w8[:, n])
                else:
                    sl = slice(c * 4, (c + 1) * 4)
                    nc.gpsimd.dma_start(wts[n][:, sl], wt[:, n, sl])

            # PE warmup: bridge the engine preamble until the first wt[0]
            # chunk lands (~11us), keeping the HAM clock busy.
            wps = pp.tile([P, NSZ], mybir.dt.float32, tag="ps", name="warmps")
            for _w in range(2 * WARMUP_MM):
                nc.tensor.matmul(wps[:, 0:NSZ // 2], warm[:, 0:P],
                                 warm[:, P:P + NSZ // 2],
                                 start=True, stop=True)

            def mm_f8(ps, m, n, colsl=None):
                """The 2 fp8 DoubleRow chunks closing the (m, n) group."""
                for j in range(NP8 // 2):
                    rhs = w8s[n][:, 2 * j:2 * j + 2, :] if colsl is None else \
                        w8s[n][:, 2 * j:2 * j + 2, colsl]
                    nc.tensor.matmul(
                        ps, x8s[m][:, 2 * j:2 * j + 2, :], rhs,
                        start=False, stop=(j == NP8 // 2 - 1), perf_mode=DR,
                    )

            def epilogue_n0(ps, m):
                ot = op_.tile([P, NSZ], mybir.dt.float32, tag="ot", name="ot")
                nc.vector.tensor_tensor(
                    ot[:], ps[:], bias_sb[:, 0:NSZ], mybir.AluOpType.add
                )
                ot16 = op_.tile([P, NSZ], mybir.dt.float16, tag="ot16", name="ot16")
                nc.vector.tensor_scalar(
                    ot16[:], ot[:], scale_sb[:, m:m + 1], 0.0,
                    mybir.AluOpType.mult, mybir.AluOpType.max,
                )
                nc.gpsimd.dma_start(y[m * P:(m + 1) * P, 0:NSZ], ot16[:])
                return ot16

            # Phase 1: n=0 sweep.  m0/m1 interleave their fp16 kd halves so
            # the matmuls consume wt0/xt chunks in delivery order during the
            # DMA ramp; m2+ run as plain accumulation groups.
            psA = pp.tile([P, NSZ], mybir.dt.float32, tag="ps", name="ps")
            psB = pp.tile([P, NSZ], mybir.dt.float32, tag="ps", name="ps")
            for kd in range(KD16 // 2):
                nc.tensor.matmul(psA[:], xts[0][:, kd], wts[0][:, kd],
                                 start=(kd == 0), stop=False)
            for kd in range(KD16 // 2):
                nc.tensor.matmul(psB[:], xts[1][:, kd], wts[0][:, kd],
                                 start=(kd == 0), stop=False)
            for kd in range(KD16 // 2, KD16):
                nc.tensor.matmul(psA[:], xts[0][:, kd], wts[0][:, kd],
                                 start=False, stop=False)
            mm_f8(psA[:], 0, 0)
            ot16 = epilogue_n0(psA, 0)
            load_wt_chunk(0, gate=ot16)
            for kd in range(KD16 // 2, KD16):
                nc.tensor.matmul(psB[:], xts[1][:, kd], wts[0][:, kd],
                                 start=False, stop=False)
            mm_f8(psB[:], 1, 0)
            ot16 = epilogue_n0(psB, 1)
            load_wt_chunk(1, gate=ot16)
            for m in range(2, m_tiles):
                ps = pp.tile([P, NSZ], mybir.dt.float32, tag="ps", name="ps")
                for kd in range(KD16):
                    nc.tensor.matmul(
                        ps[:], xts[m][:, kd], wts[0][:, kd],
                        start=(kd == 0), stop=False,
                    )
                mm_f8(ps[:], m, 0)
                ot16 = epilogue_n0(ps, m)
                if m < len(_wt_chunks):
                    load_wt_chunk(m, gate=ot16)
            for k in range(m_tiles, len(_wt_chunks)):
                load_wt_chunk(k)

            # Phase 2: m-outer / n-inner; 3 psum banks per m; epilogues
            # accumulate into one [P, 3*NSZ] fp16 tile -> single 384KB DMA
            # with 3KB lines.
            def epi_small(ps, m, n, eng):
                ot = op_.tile([P, NSZ], mybir.dt.float32, tag="ot", name="ot")
                nc.vector.tensor_tensor(
                    ot[:], ps[:], bias_sb[:, n * NSZ:(n + 1) * NSZ],
                    mybir.AluOpType.add
                )
                ot16 = op_.tile([P, NSZ], mybir.dt.float16, tag="ot16", name="ot16")
                nc.vector.tensor_scalar(
                    ot16[:], ot[:], scale_sb[:, m:m + 1], 0.0,
                    mybir.AluOpType.mult, mybir.AluOpType.max,
                )
                eng.dma_start(y[m * P:(m + 1) * P, n * NSZ:(n + 1) * NSZ], ot16[:])

            for m in range(m_tiles - 1):
                pss = [pp.tile([P, NSZ], mybir.dt.float32, tag="ps", name="ps")
                       for _ in range(NT - 1)]
                for kd in range(KD16):
                    for j in range(NT - 1):
                        nc.tensor.matmul(
                            pss[j][:], xts[m][:, kd], wts[j + 1][:, kd],
                            start=(kd == 0), stop=False,
                        )
                for p8 in range(NP8 // 2):
                    for j in range(NT - 1):
                        nc.tensor.matmul(
                            pss[j][:], x8s[m][:, 2 * p8:2 * p8 + 2, :],
                            w8s[j + 1][:, 2 * p8:2 * p8 + 2, :],
                            start=False, stop=(p8 == NP8 // 2 - 1), perf_mode=DR,
                        )
                oty = oyp.tile([P, (NT - 1) * NSZ], mybir.dt.float16, tag="oty", name="oty")
                for j in range(NT - 1):
                    n = j + 1
                    ot = op_.tile([P, NSZ], mybir.dt.float32, tag="ot", name="ot")
                    nc.vector.tensor_tensor(
                        ot[:], pss[j][:], bias_sb[:, n * NSZ:(n + 1) * NSZ],
                        mybir.AluOpType.add
                    )
                    nc.vector.tensor_scalar(
                        oty[:, j * NSZ:(j + 1) * NSZ], ot[:], scale_sb[:, m:m + 1], 0.0,
                        mybir.AluOpType.mult, mybir.AluOpType.max,
                    )
                eng = nc.sync if m % 2 == 0 else nc.gpsimd
                eng.dma_start(y[m * P:(m + 1) * P, NSZ:D], oty[:])

            # Final m-tile: per-n sequential kd loops so each n-panel's
            # epilogue + small DMA overlaps the next panel's matmuls; the
            # tail after the very last matmul is a single epilogue + 128KB.
            m = m_tiles - 1
            for j in range(NT - 1):
                if j < NT - 2:
                    ps = pp.tile([P, NSZ], mybir.dt.float32, tag="ps", name="ps")
                    for kd in range(KD16):
                        nc.tensor.matmul(
                            ps[:], xts[m][:, kd], wts[j + 1][:, kd],
                            start=(kd == 0), stop=False,
                        )
                    mm_f8(ps[:], m, j + 1)
                    epi_small(ps, m, j + 1, nc.sync)
                else:
                    # Very last panel: two sequential 256-col accumulation
                    # groups, so the first half's epilogue + DMA overlap the
                    # second half's matmuls.
                    n = j + 1
                    for c in range(2):
                        cs = slice(c * (NSZ // 2), (c + 1) * (NSZ // 2))
                        ps = pp.tile([P, NSZ], mybir.dt.float32, tag="ps", name="ps")
                        for kd in range(KD16):
                            nc.tensor.matmul(
                                ps[:, 0:NSZ // 2], xts[m][:, kd],
                                wts[n][:, kd, cs],
                                start=(kd == 0), stop=False,
                            )
                        mm_f8(ps[:, 0:NSZ // 2], m, n, colsl=cs)
                        ot = op_.tile([P, NSZ // 2], mybir.dt.float32, tag="otc", name="otc")
                        nc.vector.tensor_tensor(
                            ot[:], ps[:, 0:NSZ // 2], bias_sb[:, n * NSZ + cs.start:n * NSZ + cs.stop],
                            mybir.AluOpType.add
                        )
                        ot16 = op_.tile([P, NSZ // 2], mybir.dt.float16, tag="otc16", name="otc16")
                        nc.vector.tensor_scalar(
                            ot16[:], ot[:], scale_sb[:, m:m + 1], 0.0,
                            mybir.AluOpType.mult, mybir.AluOpType.max,
                        )
                        nc.sync.dma_start(
                            y[m * P:(m + 1) * P, n * NSZ + cs.start:n * NSZ + cs.stop],
                            ot16[:])

    nc.compile()
    return nc


def _get_nc(m_tiles):
    if m_tiles not in _nc_cache:
        _nc_cache[m_tiles] = _build(m_tiles)
    return _nc_cache[m_tiles]


def _q8(a):
    return np.clip(np.asarray(a, np.float32), -240.0, 240.0).astype(_F8)


def _prep_inputs(x, W, b, idx, vals):
    """Per-core input maps: blocked fp16/fp8 xT/wT layouts + bias/scale."""
    in_maps = []
    token_lists = []
    counts = []
    for e in range(E):
        tok = np.where((idx == e).any(axis=1))[0]
        token_lists.append(tok)
        counts.append(len(tok))
    c_max = max(counts)
    m_tiles = max(1, math.ceil(c_max / P))
    C = m_tiles * P

    for e in range(E):
        tok = token_lists[e]
        cnt = len(tok)
        Xp = np.zeros((C, D), dtype=np.float32)
        Xp[:cnt] = x[tok] * SX
        xt_np = np.ascontiguousarray(
            Xp[:, :D16].astype(_F16).reshape(m_tiles, P, KD16, P).transpose(3, 0, 2, 1)
        )
        x8_np = np.ascontiguousarray(
            _q8(Xp[:, D16:]).reshape(m_tiles, P, NP8, P).transpose(3, 0, 2, 1)
        )
        We = W[e].astype(np.float32) * SW
        wt_np = np.ascontiguousarray(
            We[:, :D16].astype(_F16).reshape(NT, NSZ, KD16, P).transpose(3, 0, 2, 1)
        )
        w8_np = np.ascontiguousarray(
            _q8(We[:, D16:]).reshape(NT, NSZ, NP8, P).transpose(3, 0, 2, 1)
        )
        bias_np = np.ascontiguousarray(
            np.broadcast_to(b[e] * SPROD, (P, D)).astype(_F16))
        s_tok = np.zeros(C, dtype=np.float32)
        for k in range(TOP_K):
            sel = idx[tok, k] == e
            s_tok[:cnt][sel] = vals[tok[sel], k]
        scale_np = np.ascontiguousarray((s_tok / SPROD).reshape(m_tiles, P).T)
        in_maps.append({"xt": xt_np, "x8": x8_np, "wt": wt_np, "w8": w8_np,
                        "bias": bias_np, "scale": scale_np})
    return in_maps, token_lists, counts, m_tiles


def kernel(x, W, b, Wg, bg):
    from concourse.bass_utils import run_bass_kernel_spmd

    x = np.asarray(x, dtype=np.float32)
    W = np.asarray(W, dtype=np.float32)
    b = np.asarray(b, dtype=np.float32)
    Wg = np.asarray(Wg, dtype=np.float32)
    bg = np.asarray(bg, dtype=np.float32)

    idx, vals = _routing(x, Wg, bg)
    in_maps, token_lists, counts, m_tiles = _prep_inputs(x, W, b, idx, vals)
    nc = _get_nc(m_tiles)
    res = run_bass_kernel_spmd(nc, in_maps, core_ids=list(range(N_CORES)))

    out = np.zeros((B, D), dtype=np.float32)
    for e in range(E):
        ye = res.results[e]["y"]
        out[token_lists[e]] += ye[:counts[e]].astype(np.float32)
    return out


# revision 15
# speedup vs baseline: 1.0158x; 1.0121x over previous
"""MoE (top-2 of 8 experts, B=8192, D=2048) on 8 Trainium2 NeuronCores.

Strategy (expert-parallel, per sharding hint): the host computes the gate
softmax + top-2 routing (float64 numpy; rank-2/3 margins are ~3e-5 so the
selection matches any f32 reference platform), dispatches each token's rows
to its experts' cores, and each core computes
    y_e = relu(x_e @ W[e].T + b[e]) * gate_scale
for its gathered tokens as a mixed fp16/fp8 tiled matmul on the PE array.
The host then scatter-adds the (at most 2) expert contributions per token.

Precision split (measured rel-err 1.6e-2 vs the 2e-2 gate): contraction
dims 0:1536 run fp16 (12 K=128 chunks); dims 1536:2048 run e4m3 fp8 as
2 DoubleRow chunks (K=256 each, 2x MACs/cycle) -> 14 streams per
(m,n)-panel instead of 16 (-12.5% PE time).  To dodge e4m3's subnormal
floor (W sigma ~0.022 < 2^-6) both planes are pre-scaled on the host
(x*2^5, W*2^10, products *2^15); the 2^-15 unscale is folded into the
pre-scaled bias (*2^15) and gate scale (*2^-15), so the epilogue is the
same two DVE ops as pure fp16.

Load balance (flex panels): expert loads are 2048 +- ~100 tokens, so
padding every core to the max (17 m-tiles = 68 panels) wastes ~6us.
Instead every core runs exactly 16 own m-tiles (64 panels) plus 2 flex
panels whose weights/tokens/bias/scale are per-core INPUTS: the 12
overflow panels (3 experts x 4 n-panels of their 17th tile) are spread
across the 16 flex slots, unused slots run zeros -> 66 panels/core.

Schedule (inherited from the fp16-only 255us kernel, at the fp16
streaming bound of 216 ns per [K128,N512] stream):
- Start: first-needed ~2.4MB interleaved across the two HWDGE queues in
  strict need-order chunks; phase 1 opens with m0/m1 interleaved at
  half-K granularity; ~20 garbage warmup matmuls bridge the ~7us engine
  preamble and hold the HAM clock gate (1.2->2.4GHz) busy.
- wt[1..3]/w8[1..3]/flex inputs are paced on the gpsimd SWDGE queue,
  each pinned behind a phase-1 epilogue output via a 1-row WAW copy
  (into the region the DMA then overwrites).
- Phase 2 runs m-outer, accumulating the n=1..3 epilogues into one
  [P,1536] tile -> a single 384KB DMA per m with 3KB lines.  The flex
  panels run last, the final one as two sequential 256-col groups so
  the post-matmul tail is one [P,256] epilogue + 128KB DMA + the fixed
  ~3us end barrier.
"""

import math

import numpy as np
import ml_dtypes

B, D, E, TOP_K = 8192, 2048, 8, 2
N_CORES = 8
P = 128
KD16 = 12          # fp16 contraction chunks (dims 0:1536)
NP8 = 4            # fp8 k-planes of 128 (dims 1536:2048) -> 2 DoubleRow chunks
D16 = KD16 * P     # 1536
NT = 4
NSZ = D // NT      # 512 output columns per psum tile
M_TILES = 16       # own tiles per core (= 2048 tokens)
FLEX = 2           # flex panel slots per core
WARMUP_MM = 10

SX = 32.0          # x pre-scale (2^5)
SW = 1024.0        # W pre-scale (2^10)
SPROD = SX * SW    # 2^15

_F16 = np.float16
_F8 = ml_dtypes.float8_e4m3   # TRN FP8_EXP4: max +-240

_nc_cache = {}


def _routing(x, Wg, bg):
    """Gate softmax + top-2 in float64; returns (idx [B,2] int, vals [B,2] f32)."""
    logits = x.astype(np.float64) @ Wg.astype(np.float64).T + bg.astype(np.float64)
    logits -= logits.max(-1, keepdims=True)
    eL = np.exp(logits)
    gate = eL / eL.sum(-1, keepdims=True)
    order = np.argsort(-gate, axis=-1, kind="stable")
    idx = order[:, :TOP_K]
    vals = np.take_along_axis(gate, idx, -1).astype(np.float32)
    return idx, vals


def _build():
    """Build + compile the per-core Bass kernel (16 own tiles + 2 flex)."""
    import concourse.mybir as mybir
    import concourse.tile as tile
    from concourse import bacc

    nc = bacc.Bacc("TRN2", target_bir_lowering=False)
    m_tiles = M_TILES
    C = m_tiles * P
    xt = nc.dram_tensor("xt", [P, m_tiles, KD16, P], mybir.dt.float16, kind="ExternalInput")
    x8 = nc.dram_tensor("x8", [P, m_tiles, NP8, P], mybir.dt.float8e4, kind="ExternalInput")
    wt = nc.dram_tensor("wt", [P, NT, KD16, NSZ], mybir.dt.float16, kind="ExternalInput")
    w8 = nc.dram_tensor("w8", [P, NT, NP8, NSZ], mybir.dt.float8e4, kind="ExternalInput")
    bias = nc.dram_tensor("bias", [P, D], mybir.dt.float16, kind="ExternalInput")
    scale = nc.dram_tensor("scale", [P, m_tiles], mybir.dt.float32, kind="ExternalInput")
    xf = nc.dram_tensor("xf", [P, FLEX, KD16, P], mybir.dt.float16, kind="ExternalInput")
    xf8 = nc.dram_tensor("xf8", [P, FLEX, NP8, P], mybir.dt.float8e4, kind="ExternalInput")
    wf = nc.dram_tensor("wf", [P, FLEX, KD16, NSZ], mybir.dt.float16, kind="ExternalInput")
    wf8 = nc.dram_tensor("wf8", [P, FLEX, NP8, NSZ], mybir.dt.float8e4, kind="ExternalInput")
    biasf = nc.dram_tensor("biasf", [P, FLEX, NSZ], mybir.dt.float16, kind="ExternalInput")
    scalef = nc.dram_tensor("scalef", [P, FLEX], mybir.dt.float32, kind="ExternalInput")
    y = nc.dram_tensor("y", [C, D], mybir.dt.float16, kind="ExternalOutput")
    yf = nc.dram_tensor("yf", [FLEX, P, NSZ], mybir.dt.float16, kind="ExternalOutput")

    DR = mybir.MatmulPerfMode.DoubleRow

    with tile.TileContext(nc) as tc:
        with (
            # Raw (dependency-untracked) SBUF for the warmup operands: the
            # garbage contents are never read back, and having no writer
            # lets the first warmup matmul issue right at PE-preamble end.
            nc.sbuf_tensor([P, 640], mybir.dt.float16) as warm,
            tc.tile_pool(name="wp", bufs=1) as wp,
            tc.tile_pool(name="w8p", bufs=1) as w8p,
            tc.tile_pool(name="xp", bufs=1) as xp,
            tc.tile_pool(name="x8p", bufs=1) as x8p,
            tc.tile_pool(name="fp", bufs=1) as fpp,
            tc.tile_pool(name="cp", bufs=1) as cp,
            tc.tile_pool(name="op", bufs=8) as op_,
            tc.tile_pool(name="oy", bufs=4) as oyp,
            tc.tile_pool(name="pp", bufs=8, space="PSUM") as pp,
        ):
            # Everything latency-critical rides the two HWDGE queues in
            # FIFO order; the gpsimd SWDGE queue stays empty until the y
            # writes (which are gated by epilogue deps) so it can never
            # starve the early loads on the shared DMA engines.
            xts = [None] * m_tiles
            x8s = [None] * m_tiles

            wts = [None] * NT
            w8s = [None] * NT
            wts[0] = wp.tile([P, KD16, NSZ], mybir.dt.float16, tag="wt0", name="wt_sb0")
            w8s[0] = w8p.tile([P, NP8, NSZ], mybir.dt.float8e4, tag="w80", name="w8_sb0")

            def load_xt_on(m, eng):
                t = xp.tile([P, KD16, P], mybir.dt.float16, tag=f"xt{m}", name=f"xt_sb{m}")
                eng.dma_start(t[:], xt[:, m])
                xts[m] = t
                t8 = x8p.tile([P, NP8, P], mybir.dt.float8e4, tag=f"x8{m}", name=f"x8_sb{m}")
                eng.dma_start(t8[:], x8[:, m])
                x8s[m] = t8

            # Both HWDGE queues carry the start-critical pieces in
            # time-of-need order (FIFO per queue); m0/m1 interleave their
            # fp16 kd halves, with the fp8 operands arriving after the
            # second halves.
            xts[0] = xp.tile([P, KD16, P], mybir.dt.float16, tag="xt0", name="xt_sb0")
            xts[1] = xp.tile([P, KD16, P], mybir.dt.float16, tag="xt1", name="xt_sb1")
            x8s[0] = x8p.tile([P, NP8, P], mybir.dt.float8e4, tag="x80", name="x8_sb0")
            x8s[1] = x8p.tile([P, NP8, P], mybir.dt.float8e4, tag="x81", name="x8_sb1")
            # SP's first packet lands ~1us before ACT's, so the two
            # first-needed pieces go one per queue, then strict
            # need-order interleave.
            nc.sync.dma_start(xts[0][:, 0:6], xt[:, 0, 0:6])
            nc.sync.dma_start(wts[0][:, 2:4], wt[:, 0, 2:4])
            nc.sync.dma_start(wts[0][:, 4:6], wt[:, 0, 4:6])
            nc.sync.dma_start(xts[0][:, 6:12], xt[:, 0, 6:12])
            nc.sync.dma_start(x8s[0][:], x8[:, 0])
            nc.sync.dma_start(w8s[0][:], w8[:, 0])

            nc.scalar.dma_start(wts[0][:, 0:2], wt[:, 0, 0:2])
            nc.scalar.dma_start(xts[1][:, 0:6], xt[:, 1, 0:6])
            nc.scalar.dma_start(wts[0][:, 6:9], wt[:, 0, 6:9])
            nc.scalar.dma_start(wts[0][:, 9:12], wt[:, 0, 9:12])
            nc.scalar.dma_start(xts[1][:, 6:12], xt[:, 1, 6:12])
            nc.scalar.dma_start(x8s[1][:], x8[:, 1])
            load_xt_on(2, nc.scalar)
            bias_sb = cp.tile([P, D], mybir.dt.float16, tag="bias", name="bias_sb")
            scale_sb = cp.tile([P, m_tiles], mybir.dt.float32, tag="scale", name="scale_sb")
            for m in range(3, m_tiles):
                load_xt_on(m, nc.sync if m % 2 == 1 else nc.scalar)
                if m == 3:
                    # bias/scale aren't needed until the first epilogue
                    # (~20us); keep them out of the critical start window.
                    nc.sync.dma_start(bias_sb[:], bias[:])
                    nc.sync.dma_start(scale_sb[:], scale[:])
            for n in range(1, NT):
                wts[n] = wp.tile([P, KD16, NSZ], mybir.dt.float16, tag=f"wt{n}", name=f"wt_sb{n}")
                w8s[n] = w8p.tile([P, NP8, NSZ], mybir.dt.float8e4, tag=f"w8{n}", name=f"w8_sb{n}")

            # flex input tiles
            xf_sb = fpp.tile([P, FLEX, KD16, P], mybir.dt.float16, tag="xf", name="xf_sb")
            xf8_sb = fpp.tile([P, FLEX, NP8, P], mybir.dt.float8e4, tag="xf8", name="xf8_sb")
            wf_sb = fpp.tile([P, FLEX, KD16, NSZ], mybir.dt.float16, tag="wf", name="wf_sb")
            wf8_sb = fpp.tile([P, FLEX, NP8, NSZ], mybir.dt.float8e4, tag="wf8", name="wf8_sb")
            biasf_sb = fpp.tile([P, FLEX, NSZ], mybir.dt.float16, tag="biasf", name="biasf_sb")
            scalef_sb = fpp.tile([P, FLEX], mybir.dt.float32, tag="scalef", name="scalef_sb")

            # Paced gpsimd loads: wt/w8 chunks for n=1..3 (4 each), then
            # flex inputs in 3 bundles.  Each is pinned behind a phase-1
            # epilogue output via a 1-row WAW copy INTO the region the DMA
            # then overwrites.
            def load_chunk(k, gate=None):
                n, c = k // 4 + 1, k % 4
                if k < 12:
                    if gate is not None:
                        if c == 3:
                            nc.vector.tensor_copy(w8s[n][0:1, 0, 0:NSZ], gate[0:1, 0:NSZ])
                        else:
                            nc.vector.tensor_copy(wts[n][0:1, 4 * c, 0:NSZ], gate[0:1, 0:NSZ])
                    if c == 3:
                        nc.gpsimd.dma_start(w8s[n][:], w8[:, n])
                    else:
                        sl = slice(c * 4, (c + 1) * 4)
                        nc.gpsimd.dma_start(wts[n][:, sl], wt[:, n, sl])
                elif k == 12:
                    if gate is not None:
                        nc.vector.tensor_copy(wf_sb[0:1, 0, 0, 0:NSZ], gate[0:1, 0:NSZ])
                    nc.gpsimd.dma_start(wf_sb[:, 0], wf[:, 0])
                    nc.gpsimd.dma_start(wf8_sb[:, 0], wf8[:, 0])
                elif k == 13:
                    if gate is not None:
                        nc.vector.tensor_copy(wf_sb[0:1, 1, 0, 0:NSZ], gate[0:1, 0:NSZ])
                    nc.gpsimd.dma_start(wf_sb[:, 1], wf[:, 1])
                    nc.gpsimd.dma_start(wf8_sb[:, 1], wf8[:, 1])
                elif k == 14:
                    if gate is not None:
                        nc.vector.tensor_copy(xf_sb[0:1, 0, 0, 0:P], gate[0:1, 0:P])
                    nc.gpsimd.dma_start(xf_sb[:], xf[:])
                    nc.gpsimd.dma_start(xf8_sb[:], xf8[:])
                    nc.gpsimd.dma_start(biasf_sb[:], biasf[:])
                    nc.gpsimd.dma_start(scalef_sb[:], scalef[:])

            N_CHUNKS = 15

            # PE warmup: bridge the engine preamble until the first wt[0]
            # chunk lands (~11us), keeping the HAM clock busy.
            wps = pp.tile([P, NSZ], mybir.dt.float32, tag="ps", name="warmps")
            for _w in range(2 * WARMUP_MM):
                nc.tensor.matmul(wps[:, 0:NSZ // 2], warm[:, 0:P],
                                 warm[:, P:P + NSZ // 2],
                                 start=True, stop=True)

            def mm_f8(ps, xt8ap, w8ap):
                """The 2 fp8 DoubleRow chunks closing a panel group."""
                for j in range(NP8 // 2):
                    nc.tensor.matmul(
                        ps, xt8ap[:, 2 * j:2 * j + 2, :], w8ap[:, 2 * j:2 * j + 2, :],
                        start=False, stop=(j == NP8 // 2 - 1), perf_mode=DR,
                    )

            def epilogue_n0(ps, m):
                ot = op_.tile([P, NSZ], mybir.dt.float32, tag="ot", name="ot")
                nc.vector.tensor_tensor(
                    ot[:], ps[:], bias_sb[:, 0:NSZ], mybir.AluOpType.add
                )
                ot16 = op_.tile([P, NSZ], mybir.dt.float16, tag="ot16", name="ot16")
                nc.vector.tensor_scalar(
                    ot16[:], ot[:], scale_sb[:, m:m + 1], 0.0,
                    mybir.AluOpType.mult, mybir.AluOpType.max,
                )
                nc.gpsimd.dma_start(y[m * P:(m + 1) * P, 0:NSZ], ot16[:])
                return ot16

            # Phase 1: n=0 sweep.  m0/m1 interleave their fp16 kd halves so
            # the matmuls consume wt0/xt chunks in delivery order during the
            # DMA ramp; m2+ run as plain accumulation groups.
            psA = pp.tile([P, NSZ], mybir.dt.float32, tag="ps", name="ps")
            psB = pp.tile([P, NSZ], mybir.dt.float32, tag="ps", name="ps")
            for kd in range(KD16 // 2):
                nc.tensor.matmul(psA[:], xts[0][:, kd], wts[0][:, kd],
                                 start=(kd == 0), stop=False)
            for kd in range(KD16 // 2):
                nc.tensor.matmul(psB[:], xts[1][:, kd], wts[0][:, kd],
                                 start=(kd == 0), stop=False)
            for kd in range(KD16 // 2, KD16):
                nc.tensor.matmul(psA[:], xts[0][:, kd], wts[0][:, kd],
                                 start=False, stop=False)
            mm_f8(psA[:], x8s[0], w8s[0])
            ot16 = epilogue_n0(psA, 0)
            load_chunk(0, gate=ot16)
            for kd in range(KD16 // 2, KD16):
                nc.tensor.matmul(psB[:], xts[1][:, kd], wts[0][:, kd],
                                 start=False, stop=False)
            mm_f8(psB[:], x8s[1], w8s[0])
            ot16 = epilogue_n0(psB, 1)
            load_chunk(1, gate=ot16)
            for m in range(2, m_tiles):
                ps = pp.tile([P, NSZ], mybir.dt.float32, tag="ps", name="ps")
                for kd in range(KD16):
                    nc.tensor.matmul(
                        ps[:], xts[m][:, kd], wts[0][:, kd],
                        start=(kd == 0), stop=False,
                    )
                mm_f8(ps[:], x8s[m], w8s[0])
                ot16 = epilogue_n0(ps, m)
                if m < N_CHUNKS:
                    load_chunk(m, gate=ot16)
            for k in range(m_tiles, N_CHUNKS):
                load_chunk(k)

            # Phase 2: m-outer / n-inner; 3 psum banks per m; epilogues
            # accumulate into one [P, 3*NSZ] fp16 tile -> single 384KB DMA
            # with 3KB lines.
            for m in range(m_tiles):
                pss = [pp.tile([P, NSZ], mybir.dt.float32, tag="ps", name="ps")
                       for _ in range(NT - 1)]
                for kd in range(KD16):
                    for j in range(NT - 1):
                        nc.tensor.matmul(
                            pss[j][:], xts[m][:, kd], wts[j + 1][:, kd],
                            start=(kd == 0), stop=False,
                        )
                for p8 in range(NP8 // 2):
                    for j in range(NT - 1):
                        nc.tensor.matmul(
                            pss[j][:], x8s[m][:, 2 * p8:2 * p8 + 2, :],
                            w8s[j + 1][:, 2 * p8:2 * p8 + 2, :],
                            start=False, stop=(p8 == NP8 // 2 - 1), perf_mode=DR,
                        )
                oty = oyp.tile([P, (NT - 1) * NSZ], mybir.dt.float16, tag="oty", name="oty")
                for j in range(NT - 1):
                    n = j + 1
                    ot = op_.tile([P, NSZ], mybir.dt.float32, tag="ot", name="ot")
                    nc.vector.tensor_tensor(
                        ot[:], pss[j][:], bias_sb[:, n * NSZ:(n + 1) * NSZ],
                        mybir.AluOpType.add
                    )
                    nc.vector.tensor_scalar(
                        oty[:, j * NSZ:(j + 1) * NSZ], ot[:], scale_sb[:, m:m + 1], 0.0,
                        mybir.AluOpType.mult, mybir.AluOpType.max,
                    )
                eng = nc.sync if m % 2 == 0 else nc.gpsimd
                eng.dma_start(y[m * P:(m + 1) * P, NSZ:D], oty[:])

            # Flex panels close the kernel.  Slot 0 runs as one [P,512]
            # group; slot 1 (the very last work) as two sequential 256-col
            # groups, so the first half's epilogue + DMA overlap the second
            # half's matmuls and the post-matmul tail is minimal.
            s = 0
            ps = pp.tile([P, NSZ], mybir.dt.float32, tag="ps", name="ps")
            for kd in range(KD16):
                nc.tensor.matmul(
                    ps[:], xf_sb[:, s, kd], wf_sb[:, s, kd],
                    start=(kd == 0), stop=False,
                )
            mm_f8(ps[:], xf8_sb[:, s], wf8_sb[:, s])
            ot = op_.tile([P, NSZ], mybir.dt.float32, tag="ot", name="ot")
            nc.vector.tensor_tensor(
                ot[:], ps[:], biasf_sb[:, s], mybir.AluOpType.add
            )
            ot16 = op_.tile([P, NSZ], mybir.dt.float16, tag="ot16", name="ot16")
            nc.vector.tensor_scalar(
                ot16[:], ot[:], scalef_sb[:, s:s + 1], 0.0,
                mybir.AluOpType.mult, mybir.AluOpType.max,
            )
            nc.sync.dma_start(yf[s], ot16[:])

            s = 1
            for c in range(2):
                cs = slice(c * (NSZ // 2), (c + 1) * (NSZ // 2))
                ps = pp.tile([P, NSZ], mybir.dt.float32, tag="ps", name="ps")
                for kd in range(KD16):
                    nc.tensor.matmul(
                        ps[:, 0:NSZ // 2], xf_sb[:, s, kd],
                        wf_sb[:, s, kd, cs],
                        start=(kd == 0), stop=False,
                    )
                for j in range(NP8 // 2):
                    nc.tensor.matmul(
                        ps[:, 0:NSZ // 2], xf8_sb[:, s, 2 * j:2 * j + 2, :],
                        wf8_sb[:, s, 2 * j:2 * j + 2, cs],
                        start=False, stop=(j == NP8 // 2 - 1), perf_mode=DR,
                    )
                otc = op_.tile([P, NSZ // 2], mybir.dt.float32, tag="otc", name="otc")
                nc.vector.tensor_tensor(
                    otc[:], ps[:, 0:NSZ // 2], biasf_sb[:, s, cs],
                    mybir.AluOpType.add
                )
                otc16 = op_.tile([P, NSZ // 2], mybir.dt.float16, tag="otc16", name="otc16")
                nc.vector.tensor_scalar(
                    otc16[:], otc[:], scalef_sb[:, s:s + 1], 0.0,
                    mybir.AluOpType.mult, mybir.AluOpType.max,
                )
                nc.sync.dma_start(yf[s, :, cs], otc16[:])

    nc.compile()
    return nc


def _get_nc():
    if 0 not in _nc_cache:
        _nc_cache[0] = _build()
    return _nc_cache[0]


def _q8(a):
    return np.clip(np.asarray(a, np.float32), -240.0, 240.0).astype(_F8)


def _xt_blocks(Xp, m_tiles):
    """[C, D] float32 (pre-scaled) -> fp16 [P,m,12,P] and fp8 [P,m,4,P]."""
    xt_np = np.ascontiguousarray(
        Xp[:, :D16].astype(_F16).reshape(m_tiles, P, KD16, P).transpose(3, 0, 2, 1)
    )
    x8_np = np.ascontiguousarray(
        _q8(Xp[:, D16:]).reshape(m_tiles, P, NP8, P).transpose(3, 0, 2, 1)
    )
    return xt_np, x8_np


def _prep_inputs(x, W, b, idx, vals):
    """Per-core input maps: blocked fp16/fp8 xT/wT layouts + flex panels."""
    token_lists = []
    counts = []
    for e in range(E):
        tok = np.where((idx == e).any(axis=1))[0]
        token_lists.append(tok)
        counts.append(len(tok))
    C = M_TILES * P

    def s_of(e, tok):
        s = np.zeros(len(tok), dtype=np.float32)
        for k in range(TOP_K):
            sel = idx[tok, k] == e
            s[sel] = vals[tok[sel], k]
        return s

    # flex jobs: overflow tiles (tokens beyond C), one job per n-panel
    flex_jobs = []  # (expert, tok_overflow, n)
    for e in range(E):
        ov = token_lists[e][C:]
        assert len(ov) <= P, "overflow beyond one tile not supported"
        if len(ov):
            for n in range(NT):
                flex_jobs.append((e, ov, n))
    assert len(flex_jobs) <= N_CORES * FLEX, "too many flex jobs"
    per_core_jobs = [[] for _ in range(N_CORES)]
    for i, job in enumerate(flex_jobs):
        per_core_jobs[i % N_CORES].append(job)

    # precompute per-expert weight blocks
    in_maps = []
    wt_cache = {}
    for e in range(E):
        We = W[e].astype(np.float32) * SW
        wt_np = np.ascontiguousarray(
            We[:, :D16].astype(_F16).reshape(NT, NSZ, KD16, P).transpose(3, 0, 2, 1)
        )
        w8_np = np.ascontiguousarray(
            We[:, D16:].reshape(NT, NSZ, NP8, P).transpose(3, 0, 2, 1)
        )
        wt_cache[e] = (wt_np, _q8(w8_np))

    for e in range(E):
        tok = token_lists[e][:C]
        cnt = len(tok)
        Xp = np.zeros((C, D), dtype=np.float32)
        Xp[:cnt] = x[tok] * SX
        xt_np, x8_np = _xt_blocks(Xp, M_TILES)
        wt_np, w8_np = wt_cache[e]
        bias_np = np.ascontiguousarray(
            np.broadcast_to(b[e] * SPROD, (P, D)).astype(_F16))
        s_tok = np.zeros(C, dtype=np.float32)
        s_tok[:cnt] = s_of(e, tok)
        scale_np = np.ascontiguousarray((s_tok / SPROD).reshape(M_TILES, P).T)

        # flex slots for this core
        xf_np = np.zeros((P, FLEX, KD16, P), dtype=_F16)
        xf8_np = np.zeros((P, FLEX, NP8, P), dtype=_F8)
        wf_np = np.zeros((P, FLEX, KD16, NSZ), dtype=_F16)
        wf8_np = np.zeros((P, FLEX, NP8, NSZ), dtype=_F8)
        biasf_np = np.zeros((P, FLEX, NSZ), dtype=_F16)
        scalef_np = np.zeros((P, FLEX), dtype=np.float32)
        for slot, (fe, ov, n) in enumerate(per_core_jobs[e]):
            Xo = np.zeros((P, D), dtype=np.float32)
            Xo[:len(ov)] = x[ov] * SX
            fxt, fx8 = _xt_blocks(Xo, 1)
            xf_np[:, slot] = fxt[:, 0]
            xf8_np[:, slot] = fx8[:, 0]
            fwt, fw8 = wt_cache[fe]
            wf_np[:, slot] = fwt[:, n]
            wf8_np[:, slot] = fw8[:, n]
            biasf_np[:, slot] = np.broadcast_to(
                (b[fe][n * NSZ:(n + 1) * NSZ] * SPROD).astype(_F16), (P, NSZ))
            so = np.zeros(P, dtype=np.float32)
            so[:len(ov)] = s_of(fe, ov)
            scalef_np[:, slot] = so / SPROD

        in_maps.append({
            "xt": xt_np, "x8": x8_np, "wt": wt_np, "w8": w8_np,
            "bias": bias_np, "scale": scale_np,
            "xf": np.ascontiguousarray(xf_np), "xf8": np.ascontiguousarray(xf8_np),
            "wf": np.ascontiguousarray(wf_np), "wf8": np.ascontiguousarray(wf8_np),
            "biasf": np.ascontiguousarray(biasf_np),
            "scalef": np.ascontiguousarray(scalef_np),
        })
    return in_maps, token_lists, counts, per_core_jobs


def kernel(x, W, b, Wg, bg):
    from concourse.bass_utils import run_bass_kernel_spmd

    x = np.asarray(x, dtype=np.float32)
    W = np.asarray(W, dtype=np.float32)
    b = np.asarray(b, dtype=np.float32)
    Wg = np.asarray(Wg, dtype=np.float32)
    bg = np.asarray(bg, dtype=np.float32)

    idx, vals = _routing(x, Wg, bg)
    in_maps, token_lists, counts, per_core_jobs = _prep_inputs(x, W, b, idx, vals)
    nc = _get_nc()
    res = run_bass_kernel_spmd(nc, in_maps, core_ids=list(range(N_CORES)))

    C = M_TILES * P
    out = np.zeros((B, D), dtype=np.float32)
    for e in range(E):
        ye = res.results[e]["y"]
        tok = token_lists[e][:C]
        out[tok] += ye[:len(tok)].astype(np.float32)
        yfe = res.results[e]["yf"]
        for slot, (fe, ov, n) in enumerate(per_core_jobs[e]):
            out[ov, n * NSZ:(n + 1) * NSZ] += yfe[slot][:len(ov)].astype(np.float32)
    return out
